# revision 1
# baseline (speedup 1.0000x reference)
"""Self-contained Trainium2 Bass kernel for the dense transformer block.

kernel(**inputs) takes the FULL unsharded fp32 inputs and returns the FULL
(B, T, C) output, distributing work across 8 NeuronCores internally.
"""
import sys as _sys
if "/opt/trn_rl_repo" not in _sys.path:
    _sys.path.insert(0, "/opt/trn_rl_repo")

from contextlib import ExitStack

import numpy as np
import ml_dtypes

import concourse.bass as bass
import concourse.tile as tile
from concourse import bacc, mybir

F32 = mybir.dt.float32
BF16 = mybir.dt.bfloat16
AF = mybir.ActivationFunctionType
ALU = mybir.AluOpType

B, T, C, H, HS, FF = 2, 2048, 1024, 16, 64, 4096
TL = 512               # local tokens per core
NCT = C // 128         # 8  c-tiles (feature tiles)
NST = T // 128         # 16 s-tiles (key token tiles)
NTC = T // TL          # 4  token chunks per batch elem
NFT = FF // 128        # 32 ff-tiles
EPS = 1e-5
N_CORES = 8
VW = HS + 1   # 65: v-column block per head = [v (64 cols) | ones]


def build_program(dbg=False):
    nc = bacc.Bacc("TRN2", target_bir_lowering=False, debug=False,
                   enable_asserts=False, num_devices=N_CORES)

    xb_d = nc.dram_tensor("xb", (C, T), BF16, kind="ExternalInput").ap()
    xloc_d = nc.dram_tensor("xloc", (C, TL), F32, kind="ExternalInput").ap()
    vmask_d = nc.dram_tensor("vmask", (T,), F32, kind="ExternalInput").ap()
    wqkv_d = nc.dram_tensor("wqkv", (C, 3 * C), BF16, kind="ExternalInput").ap()
    wproj_d = nc.dram_tensor("wproj", (C, C), BF16, kind="ExternalInput").ap()
    w1_d = nc.dram_tensor("w1", (C, FF), BF16, kind="ExternalInput").ap()
    w2_d = nc.dram_tensor("w2", (FF, C), BF16, kind="ExternalInput").ap()
    bproj_d = nc.dram_tensor("bproj", (C,), F32, kind="ExternalInput").ap()
    b1_d = nc.dram_tensor("b1", (FF,), F32, kind="ExternalInput").ap()
    b2_d = nc.dram_tensor("b2", (C,), F32, kind="ExternalInput").ap()
    ln1g_d = nc.dram_tensor("ln1g", (C,), F32, kind="ExternalInput").ap()
    ln1b_d = nc.dram_tensor("ln1b", (C,), F32, kind="ExternalInput").ap()
    ln2g_d = nc.dram_tensor("ln2g", (C,), F32, kind="ExternalInput").ap()
    ln2b_d = nc.dram_tensor("ln2b", (C,), F32, kind="ExternalInput").ap()
    out_d = nc.dram_tensor("out", (C, TL), F32, kind="ExternalOutput").ap()
    dbg_d = {}
    if dbg:
        for name, shape, dt in [
                ("dbg_xbf", (128, T), BF16), ("dbg_k", (128, T), BF16),
                ("dbg_q", (128, TL), BF16), ("dbg_v", (128, H * VW), BF16),
                ("dbg_attn", (128, TL), BF16), ("dbg_r1", (128, TL), F32),
                ("dbg_x2f", (128, TL), F32), ("dbg_r2", (128, TL), F32),
                ("dbg_mask", (128, TL), BF16)]:
            dbg_d[name] = nc.dram_tensor(name, shape, dt,
                                         kind="ExternalOutput").ap()

    with tile.TileContext(nc) as tc:
        _emit(tc, xb_d, xloc_d, vmask_d, wqkv_d, wproj_d, w1_d, w2_d,
              bproj_d, b1_d, b2_d, ln1g_d, ln1b_d, ln2g_d, ln2b_d, out_d,
              dbg_d)

    nc.compile()
    return nc


def _emit(tc, xb_d, xloc_d, vmask_d, wqkv_d, wproj_d, w1_d, w2_d, bproj_d,
          b1_d, b2_d, ln1g_d, ln1b_d, ln2g_d, ln2b_d, out_d, dbg_d={}):
    nc = tc.nc

    def dump(name, ap):
        if name in dbg_d:
            nc.sync.dma_start(dbg_d[name][:], ap)

    # ---------------- constants / small inputs ----------------
    const = tc.alloc_tile_pool(name="const", bufs=1)

    vmask_sb = const.tile([128, NST], F32, tag="vmask")          # [p, s-tile]
    nc.sync.dma_start(vmask_sb[:], vmask_d.rearrange("(a p) -> p a", p=128))

    bproj_sb = const.tile([128, NCT], F32, tag="bproj")
    nc.sync.dma_start(bproj_sb[:], bproj_d.rearrange("(a p) -> p a", p=128))
    b1_sb = const.tile([128, NFT], F32, tag="b1")
    nc.sync.dma_start(b1_sb[:], b1_d.rearrange("(a p) -> p a", p=128))
    b2_sb = const.tile([128, NCT], F32, tag="b2")
    nc.sync.dma_start(b2_sb[:], b2_d.rearrange("(a p) -> p a", p=128))
    ln1g_sb = const.tile([128, NCT], F32, tag="ln1g")
    nc.sync.dma_start(ln1g_sb[:], ln1g_d.rearrange("(a p) -> p a", p=128))
    ln1b_sb = const.tile([128, NCT], F32, tag="ln1b")
    nc.sync.dma_start(ln1b_sb[:], ln1b_d.rearrange("(a p) -> p a", p=128))
    ln2g_sb = const.tile([128, NCT], F32, tag="ln2g")
    nc.sync.dma_start(ln2g_sb[:], ln2g_d.rearrange("(a p) -> p a", p=128))
    ln2b_sb = const.tile([128, NCT], F32, tag="ln2b")
    nc.sync.dma_start(ln2b_sb[:], ln2b_d.rearrange("(a p) -> p a", p=128))

    ones_f32 = const.tile([128, 1], F32, tag="ones_f32")         # LN col-sum lhsT
    nc.gpsimd.memset(ones_f32[:], 1.0)

    eps_sb = const.tile([1, 1], F32, tag="eps")
    nc.gpsimd.memset(eps_sb[:], EPS)

    ones_big = const.tile([128, TL], BF16, tag="ones_big")       # mask source
    nc.gpsimd.memset(ones_big[:], 1.0)

    # causal masks for the 4 diagonal s-tiles (s-tiles 12..15):
    # mask_j[p, t] = 1 if j*128 + p <= t else 0
    masks = []
    for j in range(4):
        m = const.tile([128, 2 * TL], BF16, tag=f"mask{j}", name=f"mask{j}")
        for i in range(2):
            nc.gpsimd.affine_select(
                m[:, i * TL:(i + 1) * TL], ones_big[:], pattern=[[1, TL]],
                compare_op=ALU.is_ge, fill=0.0, base=-j * 128,
                channel_multiplier=-1)
        masks.append(m)

    # ---------------- phase-scoped activation storage ----------------
    # pools are LIFO stacks per side; alloc order must nest release order
    xloc_pool = tc.alloc_tile_pool(name="xloc_pool", bufs=1)     # P0..P3
    xloc = [xloc_pool.tile([128, TL], F32, tag=f"xloc{i}", name=f"xloc{i}")
            for i in range(NCT)]
    for i in range(NCT):
        nc.sync.dma_start(xloc[i][:], xloc_d[i * 128:(i + 1) * 128, :])

    kqv_pool = tc.alloc_tile_pool(name="kqv_pool", bufs=1)       # P1..P2
    k_sb = [kqv_pool.tile([128, T], BF16, tag=f"k{i}", name=f"k{i}")
            for i in range(NCT)]
    q_sb = [kqv_pool.tile([128, TL], BF16, tag=f"q{i}", name=f"q{i}")
            for i in range(NCT)]
    # v with interleaved ones-column: per s-tile [128, 16 heads x (1 + 64)]
    v_sb = [kqv_pool.tile([128, H * VW], BF16, tag=f"v{j}", name=f"v{j}")
            for j in range(NST)]

    xbf_pool = tc.alloc_tile_pool(name="xbf_pool", bufs=1)       # P0..P1
    xbf = [xbf_pool.tile([128, T], BF16, tag=f"xbf{i}", name=f"xbf{i}")
           for i in range(NCT)]
    # x fed pre-cast to bf16 from the host; split by s-chunk so the first
    # s-tile's V/K matmuls only wait for the first ~1 MB
    for sc in range(NTC):
        for i in range(NCT):
            nc.sync.dma_start(
                xbf[i][:, sc * TL:(sc + 1) * TL],
                xb_d[i * 128:(i + 1) * 128, sc * TL:(sc + 1) * TL])

    # ---------------- P1: QKV projections ----------------
    with tc.tile_pool(name="wqkv", bufs=1) as wqkv_pool:
        wqkv_sb = [wqkv_pool.tile([128, 3 * C], BF16, tag=f"wqkv{i}",
                                  name=f"wqkv{i}") for i in range(NCT)]
        for blk in (2, 1, 0):   # wv first (V matmuls start the kernel)
            for i in range(NCT):
                nc.sync.dma_start(
                    wqkv_sb[i][:, blk * C:(blk + 1) * C],
                    wqkv_d[i * 128:(i + 1) * 128, blk * C:(blk + 1) * C])

        with tc.tile_pool(name="p1psum", bufs=6, space="PSUM") as p1ps:
            # emission order matters for overlap with P2: V first (dc=0 covers
            # heads 0..7), then K+Q per d-tile so early head-pairs can start
            # attention while later projections still run.
            # V: token-major [s, d] with vmask scaling + ones columns
            for dc in range(2):
                for j in range(NST):
                    vj = v_sb[j].rearrange("p (h w) -> p h w", w=VW)
                    if dc == 0:
                        # ones columns = vmask (validity of these keys)
                        nc.gpsimd.memset(vj[:, :, HS:VW], 1.0)
                        nc.vector.tensor_scalar_mul(
                            vj[:, :, HS:VW], vj[:, :, HS:VW],
                            vmask_sb[:, j:j + 1])
                    ps = p1ps.tile([128, TL], F32, tag="p1", name="p1")
                    for c in range(NCT):
                        nc.tensor.matmul(
                            ps[:], xbf[c][:, j * 128:(j + 1) * 128],
                            wqkv_sb[c][:, 2 * C + dc * TL:2 * C + (dc + 1) * TL],
                            start=(c == 0), stop=(c == NCT - 1))
                    # psum [128 s, 512 d] = 8 heads x 64 -> strided v slots
                    h0 = dc * 8
                    nc.vector.tensor_scalar_mul(
                        vj[:, h0:h0 + 8, 0:HS],
                        ps[:].rearrange("p (h w) -> p h w", w=HS),
                        vmask_sb[:, j:j + 1])

            # K + Q per d-tile (d-tile p serves head pair p)
            for d in range(NCT):
                for sc in range(NTC):
                    ps = p1ps.tile([128, TL], F32, tag="p1", name="p1")
                    for c in range(NCT):
                        nc.tensor.matmul(
                            ps[:], wqkv_sb[c][:, C + d * 128:C + (d + 1) * 128],
                            xbf[c][:, sc * TL:(sc + 1) * TL],
                            start=(c == 0), stop=(c == NCT - 1))
                    nc.vector.tensor_copy(k_sb[d][:, sc * TL:(sc + 1) * TL], ps[:])
                ps = p1ps.tile([128, TL], F32, tag="p1", name="p1")
                for c in range(NCT):
                    nc.tensor.matmul(
                        ps[:], wqkv_sb[c][:, d * 128:(d + 1) * 128],
                        xbf[c][:, T - TL:T],
                        start=(c == 0), stop=(c == NCT - 1))
                nc.scalar.activation(q_sb[d][:], ps[:], AF.Copy)

    dump("dbg_xbf", xbf[0][:])
    dump("dbg_k", k_sb[0][:])
    dump("dbg_q", q_sb[0][:])
    dump("dbg_v", v_sb[0][:])
    dump("dbg_mask", masks[0][:])
    xbf_pool.release()

    # wproj loads (used in P3; issue now so DMA overlaps attention)
    wproj_pool = tc.alloc_tile_pool(name="wproj", bufs=1, side="right")
    wproj_sb = [wproj_pool.tile([128, C], BF16, tag=f"wp{i}", name=f"wp{i}")
                for i in range(NCT)]
    for i in range(NCT):
        nc.sync.dma_start(wproj_sb[i][:], wproj_d[i * 128:(i + 1) * 128, :])

    attn_pool = tc.alloc_tile_pool(name="attn_pool", bufs=1,
                                   side="right")                 # P2..P3
    attn_sb = [attn_pool.tile([128, TL], BF16, tag=f"attn{i}", name=f"attn{i}")
               for i in range(NCT)]

    # ---------------- P2: attention ----------------
    # heads processed in pairs: the two score matmuls go to different PE row
    # groups (partition offsets 0/64) and run concurrently; one double-width
    # exp on ScalarE covers both heads' scores.
    with tc.tile_pool(name="p2sbuf", bufs=1) as p2sb, \
         tc.tile_pool(name="p2psum", bufs=1, space="PSUM") as p2ps:
        # preload the Exp table while P1 still computes
        warm = p2sb.tile([1, 1], F32, tag="warm", name="warm")
        nc.scalar.activation(warm[:], eps_sb[:], AF.Exp)
        for pair in range(H // 2):
            h0, h1 = 2 * pair, 2 * pair + 1
            avs = [p2ps.tile([VW, TL], F32, tag="av", bufs=4, name="av")
                   for _ in range(2)]
            for j in range(NST):
                sc = p2ps.tile([128, 2 * TL], F32, tag="sc", bufs=2, name="sc")
                for i, po in ((0, 0), (1, 64)):
                    nc.tensor.matmul(
                        sc[:, i * TL:(i + 1) * TL],
                        k_sb[pair][po:po + HS, j * 128:(j + 1) * 128],
                        q_sb[pair][po:po + HS, :],
                        start=True, stop=True, tile_position=(po, 0))
                probs = p2sb.tile([128, 2 * TL], BF16, tag="probs", bufs=6,
                                  name="probs")
                nc.scalar.activation(probs[:], sc[:], AF.Exp,
                                     scale=float(HS) ** -0.5)
                if j >= NST - 4:
                    m = masks[j - (NST - 4)]
                    nc.vector.tensor_mul(probs[:], probs[:], m[:])
                for i, h in ((0, h0), (1, h1)):
                    nc.tensor.matmul(
                        avs[i][:], v_sb[j][:, h * VW:(h + 1) * VW],
                        probs[:, i * TL:(i + 1) * TL],
                        start=(j == 0), stop=(j == NST - 1))
            for i, h in ((0, h0), (1, h1)):
                av = avs[i]
                po = (h % 2) * 64
                # av rows 0..63 = unnormalized attn, row 64 = denominator
                rcp = p2sb.tile([VW, TL], F32, tag="rcp", bufs=2, name="rcp")
                nc.vector.reciprocal(rcp[HS:VW, :], av[HS:VW, :])
                # partition_broadcast only reads partition base 0 on HW ->
                # move the reciprocal row from partition 64 to 0 via DMA
                rcp0 = p2sb.tile([1, TL], F32, tag="rcp0", bufs=2, name="rcp0")
                nc.sync.dma_start(rcp0[:], rcp[HS:VW, :])
                bc = p2sb.tile([HS, TL], F32, tag="bc", bufs=2, name="bc")
                nc.gpsimd.partition_broadcast(bc[:], rcp0[:])
                if po == 0:
                    # even head: write normalized attn straight to its slot
                    nc.vector.tensor_mul(attn_sb[pair][0:HS, :],
                                         av[0:HS, :], bc[:])
                else:
                    atmp = p2sb.tile([HS, TL], BF16, tag="atmp", bufs=2,
                                     name="atmp")
                    nc.vector.tensor_mul(atmp[:], av[0:HS, :], bc[:])
                    nc.sync.dma_start(attn_sb[pair][po:po + HS, :], atmp[:])

    dump("dbg_attn", attn_sb[0][:])
    kqv_pool.release()

    # ---------------- P3: output projection + residual + LN1 ----------------
    with tc.tile_pool(name="p3sbuf", bufs=1) as p3sb, \
         tc.tile_pool(name="p3psum", bufs=1, space="PSUM") as p3ps:
        resid1 = [p3sb.tile([128, TL], F32, tag=f"r1_{i}", name=f"r1_{i}")
                  for i in range(NCT)]
        for e in range(NCT):
            ps = p3ps.tile([128, TL], F32, tag="pr", bufs=3, name="pr")
            for d in range(NCT):
                nc.tensor.matmul(
                    ps[:], wproj_sb[d][:, e * 128:(e + 1) * 128], attn_sb[d][:],
                    start=(d == 0), stop=(d == NCT - 1))
            sa = p3sb.tile([128, TL], F32, tag="sa", bufs=2, name="sa")
            nc.scalar.activation(sa[:], ps[:], AF.Identity,
                                 bias=bproj_sb[:, e:e + 1])
            nc.vector.tensor_add(resid1[e][:], sa[:], xloc[e][:])

        dump("dbg_r1", resid1[0][:])
        attn_pool.release()
        wproj_pool.release()

        x2_pool = tc.alloc_tile_pool(name="x2_pool", bufs=1,
                                     side="right")               # P3..P4
        x2f = [x2_pool.tile([128, TL], F32, tag=f"x2f{i}", name=f"x2f{i}")
               for i in range(NCT)]
        x2b = [x2_pool.tile([128, TL], BF16, tag=f"x2b{i}", name=f"x2b{i}")
               for i in range(NCT)]

        # FFN weight pool + first-half prefetch (overlaps LN1)
        p4w = tc.alloc_tile_pool(name="p4w_pool", bufs=1, side="right")
        w1h = [p4w.tile([128, FF // 2], BF16, tag=f"w1h{i}", bufs=1,
                        name=f"w1h{i}") for i in range(NCT)]
        for i in range(NCT):
            nc.sync.dma_start(w1h[i][:], w1_d[i * 128:(i + 1) * 128,
                                              0:(NFT // 2) * 128])
        w2q0 = [p4w.tile([128, C], BF16, tag=f"w2q{i}", bufs=2,
                         name=f"w2q{i}") for i in range(NFT // 4)]
        for i in range(NFT // 4):
            nc.sync.dma_start(w2q0[i][:], w2_d[i * 128:(i + 1) * 128, :])

        _layernorm(tc, p3sb, p3ps, resid1, ln1g_sb, ln1b_sb, ones_f32,
                   eps_sb, out_f32=x2f, out_bf16=x2b)

        dump("dbg_x2f", x2f[0][:])
    xloc_pool.release()

    # ---------------- P4: FFN ----------------
    r2_pool = tc.alloc_tile_pool(name="r2_pool", bufs=1)         # P4..P5
    resid2 = [r2_pool.tile([128, TL], F32, tag=f"r2_{i}", name=f"r2_{i}")
              for i in range(NCT)]

    with tc.tile_pool(name="p4sbuf", bufs=1) as p4sb, \
         tc.tile_pool(name="p4psum", bufs=1, space="PSUM") as p4ps:
        for half in range(2):
            f0 = half * (NFT // 2)
            if half == 1:
                w1h = [p4w.tile([128, FF // 2], BF16, tag=f"w1h{i}", bufs=1,
                                name=f"w1h{i}") for i in range(NCT)]
                for i in range(NCT):
                    nc.sync.dma_start(
                        w1h[i][:], w1_d[i * 128:(i + 1) * 128,
                                        f0 * 128:(f0 + NFT // 2) * 128])
            h_sb = [p4sb.tile([128, TL], BF16, tag=f"h{i}", bufs=2,
                              name=f"h{i}") for i in range(NFT // 2)]
            # FFN1 + fused bias+relu evac
            for fi in range(NFT // 2):
                f = f0 + fi
                ps = p4ps.tile([128, TL], F32, tag="h1", bufs=3, name="h1")
                for c in range(NCT):
                    nc.tensor.matmul(
                        ps[:], w1h[c][:, fi * 128:(fi + 1) * 128], x2b[c][:],
                        start=(c == 0), stop=(c == NCT - 1))
                nc.vector.tensor_scalar(
                    h_sb[fi][:], ps[:], b1_sb[:, f:f + 1], 0.0,
                    op0=ALU.add, op1=ALU.max)
            # FFN2, w2 streamed in quarters
            for quarter in range(2):
                fq0 = quarter * (NFT // 4)
                if half == 0 and quarter == 0:
                    w2q = w2q0
                else:
                    w2q = [p4w.tile([128, C], BF16, tag=f"w2q{i}", bufs=2,
                                    name=f"w2q{i}") for i in range(NFT // 4)]
                    for i in range(NFT // 4):
                        f = f0 + fq0 + i
                        nc.sync.dma_start(w2q[i][:],
                                          w2_d[f * 128:(f + 1) * 128, :])
                for e in range(NCT):
                    ps = p4ps.tile([128, TL], F32, tag="ff", bufs=3, name="ff")
                    for i in range(NFT // 4):
                        nc.tensor.matmul(
                            ps[:], w2q[i][:, e * 128:(e + 1) * 128],
                            h_sb[fq0 + i][:],
                            start=(i == 0), stop=(i == NFT // 4 - 1))
                    if half == 0 and quarter == 0:
                        tmp = p4sb.tile([128, TL], F32, tag="ft", bufs=2,
                                        name="ft")
                        nc.scalar.activation(tmp[:], ps[:], AF.Identity,
                                             bias=b2_sb[:, e:e + 1])
                        nc.vector.tensor_add(resid2[e][:], tmp[:], x2f[e][:])
                    else:
                        nc.vector.tensor_add(resid2[e][:], resid2[e][:], ps[:])

    p4w.release()
    x2_pool.release()

    # ---------------- LN2 + output ----------------
    with tc.tile_pool(name="p5sbuf", bufs=1) as p5sb, \
         tc.tile_pool(name="p5psum", bufs=1, space="PSUM") as p5ps:
        of = [p5sb.tile([128, TL], F32, tag=f"of{i}", name=f"of{i}")
              for i in range(NCT)]
        _layernorm(tc, p5sb, p5ps, resid2, ln2g_sb, ln2b_sb, ones_f32,
                   eps_sb, out_f32=of, out_bf16=None)
        for i in range(NCT):
            nc.sync.dma_start(out_d[i * 128:(i + 1) * 128, :], of[i][:])

    r2_pool.release()
    const.release()


def _layernorm(tc, sb_pool, ps_pool, resid, g_sb, b_sb, ones, eps_sb,
               out_f32, out_bf16):
    """Feature-major LayerNorm over the partition (feature) axis.

    resid: 8 tiles [128, TL] fp32. Writes g*xhat+b into out_f32 (and a bf16
    copy into out_bf16 when given).
    """
    nc = tc.nc
    mu_ps = ps_pool.tile([1, TL], F32, tag="mu", bufs=2, name="mu")
    sq_ps = ps_pool.tile([1, TL], F32, tag="sq", bufs=2, name="sq")
    for i in range(NCT):
        nc.tensor.matmul(mu_ps[:], ones[:], resid[i][:],
                         start=(i == 0), stop=(i == NCT - 1))
    for i in range(NCT):
        sqt_i = sb_pool.tile([128, TL], F32, tag="sqt", bufs=2, name="sqt")
        nc.scalar.square(sqt_i[:], resid[i][:])
        nc.tensor.matmul(sq_ps[:], ones[:], sqt_i[:],
                         start=(i == 0), stop=(i == NCT - 1))

    mu = sb_pool.tile([1, TL], F32, tag="lnmu", bufs=2, name="lnmu")
    nc.vector.tensor_scalar_mul(mu[:], mu_ps[:], 1.0 / C)
    ms = sb_pool.tile([1, TL], F32, tag="lnms", bufs=2, name="lnms")
    nc.vector.tensor_scalar_mul(ms[:], sq_ps[:], 1.0 / C)
    mu2 = sb_pool.tile([1, TL], F32, tag="lnmu2", bufs=2, name="lnmu2")
    nc.vector.tensor_mul(mu2[:], mu[:], mu[:])
    var = sb_pool.tile([1, TL], F32, tag="lnvar", bufs=2, name="lnvar")
    nc.vector.tensor_sub(var[:], ms[:], mu2[:])
    sd = sb_pool.tile([1, TL], F32, tag="lnsd", bufs=2, name="lnsd")
    nc.scalar.activation(sd[:], var[:], AF.Sqrt, bias=eps_sb[:])
    rstd = sb_pool.tile([1, TL], F32, tag="lnrstd", bufs=2, name="lnrstd")
    nc.vector.reciprocal(rstd[:], sd[:])

    mu_bc = sb_pool.tile([128, TL], F32, tag="lnmubc", bufs=2, name="lnmubc")
    nc.gpsimd.partition_broadcast(mu_bc[:], mu[:])
    rs_bc = sb_pool.tile([128, TL], F32, tag="lnrsbc", bufs=2, name="lnrsbc")
    nc.gpsimd.partition_broadcast(rs_bc[:], rstd[:])

    for i in range(NCT):
        t1 = sb_pool.tile([128, TL], F32, tag="lnt1", bufs=2, name="lnt1")
        nc.vector.tensor_sub(t1[:], resid[i][:], mu_bc[:])
        t2 = sb_pool.tile([128, TL], F32, tag="lnt2", bufs=2, name="lnt2")
        nc.vector.tensor_mul(t2[:], t1[:], rs_bc[:])
        nc.vector.tensor_scalar(out_f32[i][:], t2[:], g_sb[:, i:i + 1],
                                b_sb[:, i:i + 1], op0=ALU.mult, op1=ALU.add)
        if out_bf16 is not None:
            nc.vector.tensor_copy(out_bf16[i][:], out_f32[i][:])


# ---------------- host side ----------------

def host_prepare(x, wq, wk, wv, wproj, bproj, ln1_g, ln1_b, w1, b1, w2, b2,
                 ln2_g, ln2_b):
    """Build the 8 per-core input maps from full fp32 inputs."""
    bf = ml_dtypes.bfloat16
    # weights: [in, out] layout == matmul lhsT
    wq2 = np.ascontiguousarray(wq.transpose(1, 0, 2).reshape(C, C))
    wk2 = np.ascontiguousarray(wk.transpose(1, 0, 2).reshape(C, C))
    wv2 = np.ascontiguousarray(wv.transpose(1, 0, 2).reshape(C, C))
    wqkv = np.concatenate([wq2, wk2, wv2], axis=1).astype(bf)
    shared = {
        "wqkv": wqkv,
        "wproj": wproj.astype(bf),
        "w1": w1.astype(bf),
        "w2": w2.astype(bf),
        "bproj": bproj.astype(np.float32),
        "b1": b1.astype(np.float32),
        "b2": b2.astype(np.float32),
        "ln1g": ln1_g.astype(np.float32),
        "ln1b": ln1_b.astype(np.float32),
        "ln2g": ln2_g.astype(np.float32),
        "ln2b": ln2_b.astype(np.float32),
    }
    in_maps = []
    for core in range(N_CORES):
        b, c = divmod(core, NTC)
        shift = (3 - c) * TL
        xT = np.ascontiguousarray(np.roll(x[b].T, shift, axis=1)).astype(
            np.float32)                      # [C, T], rotated
        gidx = (np.arange(T) - shift) % T    # global token of rotated col i
        vmask = (gidx < (c + 1) * TL).astype(np.float32)
        in_maps.append({"xb": xT.astype(bf), "xloc": xT[:, T - TL:T].copy(),
                        "vmask": vmask, **shared})
    return in_maps


def host_finalize(results):
    """Assemble full (B, T, C) output from per-core [C, TL] feature-major."""
    out = np.empty((B, T, C), np.float32)
    for core in range(N_CORES):
        b, c = divmod(core, NTC)
        out[b, c * TL:(c + 1) * TL, :] = results[core]["out"].T
    return out


# ---------------- top-level entry ----------------
from concourse.bass_utils import run_bass_kernel_spmd as _run_spmd

_nc_cache = None


def _program():
    global _nc_cache
    if _nc_cache is None:
        _nc_cache = build_program()
    return _nc_cache


def run(inputs, trace=False):
    nc = _program()
    in_maps = host_prepare(**inputs)
    res = _run_spmd(nc, in_maps, core_ids=list(range(N_CORES)), trace=trace)
    return host_finalize(res.results), res


def kernel(**inputs):
    out, _ = run(inputs, trace=False)
    return out

